# revision 22
# baseline (speedup 1.0000x reference)
"""Trainium2 Bass kernel for nn_BatchMultiHeadGraphAttention.

Math: out[b,c,h] = softmax_j(mask_adj(leaky_relu(src_i + dst_j))) @ Hm where
Hm = h[b,c] @ w[c,h], t = tanh(Hm), src = t @ a_src, dst = t @ a_dst.

Key identity: exp(leaky(s)) = max(e^s, e^{0.2 s}).  The row factor
e^{0.2 src_i} cancels in softmax normalization, leaving

  P_ij  propto_i  v_ij * W_ij,   W_ij = max(sig_i * b_j, d_j)

with sig = e^{0.8 src}, b = e^{dst}, d = e^{0.2 dst}.  Per (c,h) unit the
N^2 work is exactly: 8 tensor_scalar ops (op0=mult by per-partition b_j,
op1=max with d_j) on a broadcast tile of sig, one tensor_tensor multiply
with the transposed adjacency (both DVE), and one 65-column matmul chain
num = (v .* W)^T @ [Hm | 1]; then out = num[:, :64] * (1/num[:, 64]).

The adjacency (with self-loops) is transposed/bf16-packed on the host;
all exp/tanh/matmul compute runs on device.  dst is produced directly in
column layout by small matmuls (t-blocks as lhsT); sigma rows are
broadcast across partitions via a ones-vector matmul for the first group
(latency critical) and via DRAM round-trip DMA broadcast for the rest.
Prep for group k+1 is interleaved between the mask/matmul pipelines of
units of group k; epilogues are emitted behind the next unit's masks so
the vector engine never stalls.

Sharding: core = b*2 + cpair; each core does one b and two c's (all 4
heads), data-parallel over the leading batch dim as hinted.
"""

import sys
from contextlib import ExitStack

import numpy as np
import ml_dtypes

sys.path.insert(0, "/opt/trn_rl_repo")

import concourse.bass as bass
import concourse.bacc as bacc
import concourse.tile as tile
from concourse import mybir
from concourse.bass_utils import run_bass_kernel_spmd

F32 = mybir.dt.float32
BF16 = mybir.dt.bfloat16
AF = mybir.ActivationFunctionType
OP = mybir.AluOpType

N = 1024
NB = 8  # 128-row blocks
F = 64
C2 = 2  # c's per core
NH = 4  # heads


def build_kernel(nc: bass.Bass, tc: tile.TileContext, ctx: ExitStack, ins, out_ap):
    vT_ap = ins["vT"]
    hTb_ap = ins["hTb"]
    w4_ap = ins["w4"]
    wb2_ap = ins["wb2"]
    aabd_ap = ins["aabd"]

    # ---------------- pools ----------------
    constp = ctx.enter_context(tc.tile_pool(name="const", bufs=1))
    prepp = ctx.enter_context(tc.tile_pool(name="prepp", bufs=2))
    sigbp = ctx.enter_context(tc.tile_pool(name="sigbp", bufs=8))
    vppool = ctx.enter_context(tc.tile_pool(name="vppool", bufs=3))
    vecp = ctx.enter_context(tc.tile_pool(name="vecp", bufs=4))
    outp = ctx.enter_context(tc.tile_pool(name="outp", bufs=2))
    psprep = ctx.enter_context(tc.tile_pool(name="psprep", bufs=2, space="PSUM"))
    pspp = ctx.enter_context(tc.tile_pool(name="pspp", bufs=4, space="PSUM"))

    # ---------------- constants in (small tensors first, vT last) ---------
    wb2_sb = constp.tile([64, C2, 2, 128], BF16)
    nc.sync.dma_start(out=wb2_sb[:], in_=wb2_ap[:])
    aabd_sb = constp.tile([128, C2, 2, 4], BF16)
    nc.sync.dma_start(out=aabd_sb[:], in_=aabd_ap[:])
    hTb0 = constp.tile([64, N], BF16)
    nc.sync.dma_start(out=hTb0[:, 0:512], in_=hTb_ap[0:64, 0:512])
    nc.sync.dma_start(out=hTb0[:, 512:1024], in_=hTb_ap[0:64, 512:1024])
    hTb1 = constp.tile([64, N], BF16)
    nc.sync.dma_start(out=hTb1[:], in_=hTb_ap[64:128, :])
    hTb_t = [hTb0, hTb1]
    w4_sb = constp.tile([64, C2, NH * F], BF16)
    nc.sync.dma_start(out=w4_sb[:], in_=w4_ap[:])
    vT = constp.tile([128, NB, N], BF16)
    nc.sync.dma_start(out=vT[:], in_=vT_ap[:].rearrange("p (nb n) -> p nb n", nb=NB))

    ones1 = constp.tile([1, 128], BF16)
    nc.vector.memset(ones1[:], 1.0)
    dramp = ctx.enter_context(tc.tile_pool(name="dramp", bufs=1, space="DRAM"))
    sig_d = dramp.tile([8, N], BF16, tag="sigd")
    bcols = constp.tile([128, 8, NB], F32)  # e^{dst_j}; unit index ch = c*4+h
    dcols = constp.tile([128, 8, NB], F32)  # e^{0.2 dst_j}
    sigb = {}  # per unit broadcast tiles [128, N] of e^{0.8 src_i}

    # Haug per c: [128, nb, c, h, 65] bf16 (col 64 = ones)
    haug = constp.tile([128, NB, C2, NH, 65], BF16)
    nc.vector.memset(haug[:, :, :, :, 64:65], 1.0)

    def prep_group(c, hp, pe_bcast=False):
        tT = prepp.tile([128, 2, 512], BF16, tag="tT")
        sigh = [
            prepp.tile([1, N], BF16, tag="sigh", name=f"sigh{c}{hp}0"),
            prepp.tile([1, N], BF16, tag="sigh2", name=f"sigh{c}{hp}1"),
        ]
        sbs = [
            sigbp.tile([128, N], BF16, tag="sigb", name=f"sigb{c*4+2*hp}"),
            sigbp.tile([128, N], BF16, tag="sigb", name=f"sigb{c*4+2*hp+1}"),
        ]
        for half in range(2):
            hs = slice(half * 512, (half + 1) * 512)
            ps_t = psprep.tile([128, 512], F32, tag="prep")
            nc.tensor.matmul(
                ps_t[:],
                lhsT=wb2_sb[:, c, hp, :],
                rhs=hTb_t[c][:, hs],
                start=True,
                stop=True,
            )
            nc.scalar.activation(out=tT[:, half, :], in_=ps_t[:], func=AF.Tanh)
        # dst columns directly via matmul: psd[:, jb*2+hh] = dst^{head hh}_j
        psd = psprep.tile([128, 16], F32, tag="psd")
        for jb in range(NB):
            half, q = jb // 4, jb % 4
            nc.tensor.matmul(
                psd[:, jb * 2:jb * 2 + 2],
                lhsT=tT[:, half, q * 128:(q + 1) * 128],
                rhs=aabd_sb[:, c, hp, 2:4],
                start=True,
                stop=True,
            )
        for hh in range(2):
            ch = c * 4 + 2 * hp + hh
            for half in range(2):
                hs = slice(half * 512, (half + 1) * 512)
                ps_src = psprep.tile([128, 512], F32, tag="prep")
                nc.tensor.matmul(
                    ps_src[0:1, :],
                    lhsT=aabd_sb[:, c, hp, hh:hh + 1],
                    rhs=tT[:, half, :],
                    start=True,
                    stop=True,
                )
                nc.scalar.activation(
                    out=sigh[hh][:, hs], in_=ps_src[0:1, :], func=AF.Exp, scale=0.8
                )
                if pe_bcast:
                    # broadcast sigma row across partitions on the PE
                    psb = psprep.tile([128, 512], F32, tag="prep")
                    nc.tensor.matmul(
                        psb[:],
                        lhsT=ones1[:],
                        rhs=sigh[hh][:, hs],
                        start=True,
                        stop=True,
                    )
                    nc.scalar.activation(
                        out=sbs[hh][:, hs], in_=psb[:], func=AF.Copy
                    )
            dview = psd[:, 0:16].rearrange("p (jb h) -> p h jb", h=2)[:, hh, :]
            nc.scalar.activation(out=bcols[:, ch, :], in_=dview, func=AF.Exp)
            nc.scalar.activation(
                out=dcols[:, ch, :], in_=dview, func=AF.Exp, scale=0.2
            )
            if not pe_bcast:
                nc.sync.dma_start(out=sig_d[ch:ch + 1, :], in_=sigh[hh][:])
                nc.gpsimd.dma_start(
                    out=sbs[hh][:],
                    in_=sig_d[ch:ch + 1, :].to_broadcast([128, N]),
                )
            sigb[ch] = sbs[hh]

    def prep_haug(c):
        for nb in range(NB):
            ph = psprep.tile([128, 512], F32, tag="prep")
            nc.tensor.matmul(
                ph[:, 0:256],
                lhsT=hTb_t[c][:, nb * 128:(nb + 1) * 128],
                rhs=w4_sb[:, c, :],
                start=True,
                stop=True,
            )
            nc.scalar.activation(
                out=haug[:, nb, c, :, 0:64],
                in_=ph[:, 0:256].rearrange("p (h o) -> p h o", h=NH),
                func=AF.Copy,
            )


    # ---------------- N^2 phase, software-pipelined ----------------
    UNITS = [(0, 0), (0, 1), (1, 0), (1, 1), (0, 2), (0, 3), (1, 2), (1, 3)]

    state = {}

    def masks(idx):
        c, h = UNITS[idx]
        ch = c * 4 + h
        W = vppool.tile([128, NB, N], BF16, tag="wv", name=f"wv{ch}")
        for jb in range(NB):
            nc.vector.tensor_scalar(
                out=W[:, jb, :],
                in0=sigb[ch][:],
                scalar1=bcols[:, ch, jb:jb + 1],
                scalar2=dcols[:, ch, jb:jb + 1],
                op0=OP.mult,
                op1=OP.max,
            )
        if idx == len(UNITS) - 1:
            for ihalf in range(2):
                cols = slice(ihalf * 512, (ihalf + 1) * 512)
                nc.vector.tensor_tensor(
                    out=W[:, :, cols], in0=W[:, :, cols], in1=vT[:, :, cols],
                    op=OP.mult,
                )
        else:
            nc.vector.tensor_tensor(out=W[:], in0=W[:], in1=vT[:], op=OP.mult)
        state[idx] = W

    def chains(idx):
        c, h = UNITS[idx]
        W = state[idx]
        pss = []
        for ihalf in range(2):
            ps_n = pspp.tile([128, 4, 65], F32, tag="psn")
            for q in range(4):
                ib = ihalf * 4 + q
                for jb in range(NB):
                    nc.tensor.matmul(
                        ps_n[:, q, :],
                        lhsT=W[:, jb, ib * 128:(ib + 1) * 128],
                        rhs=haug[:, jb, c, h, :],
                        start=(jb == 0),
                        stop=(jb == NB - 1),
                    )
            pss.append(ps_n)
        state[idx] = (pss, c, h)

    def epilogue(idx):
        pss, c, h = state.pop(idx)
        outb = outp.tile([128, NB, F], BF16, tag="outb")
        for ihalf in range(2):
            ps_n = pss[ihalf]
            rec = vecp.tile([128, 4, 1], F32, tag="rec")
            nc.vector.reciprocal(out=rec[:], in_=ps_n[:, :, 64:65])
            for q in range(4):
                ib = ihalf * 4 + q
                nc.scalar.activation(
                    out=outb[:, ib, :],
                    in_=ps_n[:, q, 0:64],
                    func=AF.Copy,
                    scale=rec[:, q, :],
                )
        for ihalf in range(2):
            nc.sync.dma_start(
                out=out_ap[c, h, ihalf * 512:(ihalf + 1) * 512, :].rearrange(
                    "(ib p) o -> p ib o", p=128
                ),
                in_=outb[:, ihalf * 4:(ihalf + 1) * 4, :],
            )

    prep_group(0, 0, pe_bcast=True)
    masks(0)
    prep_haug(0)
    prep_group(1, 0, pe_bcast=True)
    chains(0)
    masks(1)
    prep_haug(1)
    chains(1)
    epilogue(0)
    prep_group(0, 1)
    masks(2)
    chains(2)
    epilogue(1)
    prep_group(1, 1)
    for idx in range(3, len(UNITS) - 1):
        masks(idx)
        chains(idx)
        epilogue(idx - 1)
    masks(7)
    epilogue(6)
    chains(7)
    epilogue(7)


def _install_ntff_hook():
    """antenv.axon_hooks is missing in this image; inject an equivalent shim
    driving NTFF profiling via ctypes into libaxon_pjrt.so."""
    import types, ctypes, contextlib

    if "antenv.axon_hooks" in sys.modules:
        return
    so_path = "/opt/axon/libaxon_pjrt.so"
    try:
        lib = ctypes.CDLL(so_path)
        lib.axon_start_nrt_profile.argtypes = [
            ctypes.POINTER(ctypes.c_int64),
            ctypes.c_size_t,
        ]
        lib.axon_start_nrt_profile.restype = ctypes.c_int64
        lib.axon_stop_nrt_profile.argtypes = [ctypes.c_char_p]
        lib.axon_stop_nrt_profile.restype = ctypes.c_int64
    except (OSError, AttributeError):
        return

    @contextlib.contextmanager
    def _hook(output_dir, device_ids):
        import jax

        jax.devices()
        if device_ids:
            ids = (ctypes.c_int64 * len(device_ids))(*device_ids)
            rc = lib.axon_start_nrt_profile(ids, len(device_ids))
        else:
            rc = lib.axon_start_nrt_profile(None, 0)
        if rc != 0:
            raise RuntimeError(f"axon_start_nrt_profile rc={rc}")
        try:
            yield
        finally:
            n = lib.axon_stop_nrt_profile(str(output_dir).encode())
            print(f"profile: {n} file(s) written to {output_dir}", file=sys.stderr)

    mod = types.ModuleType("antenv.axon_hooks")
    mod.get_axon_ntff_profile_hook = lambda: _hook
    mod.set_axon_ntff_profile_hook = lambda h: None
    sys.modules["antenv.axon_hooks"] = mod

    import concourse.bass_utils as bu

    bu.upload_artifacts = lambda tmpdir: f"local:{tmpdir}"


_CACHED = {}


def _build_program():
    if "nc" in _CACHED:
        return _CACHED["nc"]
    nc = bacc.Bacc(
        "TRN2",
        target_bir_lowering=False,
        debug=False,
        enable_asserts=True,
        num_devices=8,
    )
    ins = {
        "vT": nc.dram_tensor("vT", [128, NB * N], BF16, kind="ExternalInput").ap(),
        "hTb": nc.dram_tensor("hTb", [128, N], BF16, kind="ExternalInput").ap(),
        "w4": nc.dram_tensor("w4", [64, C2, NH * F], BF16, kind="ExternalInput").ap(),
        "wb2": nc.dram_tensor("wb2", [64, C2, 2, 128], BF16, kind="ExternalInput").ap(),
        "aabd": nc.dram_tensor("aabd", [128, C2, 2, 4], BF16, kind="ExternalInput").ap(),
    }
    out_ap = nc.dram_tensor(
        "out_loc", [C2, NH, N, F], BF16, kind="ExternalOutput"
    ).ap()
    with tile.TileContext(nc) as tc:
        with ExitStack() as ctx:
            build_kernel(nc, tc, ctx, ins, out_ap)
    nc.compile()
    _CACHED["nc"] = nc
    return nc


def make_in_maps(h, adj, w, a_src, a_dst):
    bf = ml_dtypes.bfloat16
    in_maps = []
    eye = np.eye(N, dtype=np.float32)
    for core in range(8):
        b, cp = core // 2, core % 2
        cs = slice(2 * cp, 2 * cp + 2)
        # vT[p, jb, i] = (adj[b] or I)[i, jb*128+p]
        adjsl = ((adj[b] + eye) != 0).astype(np.float32)
        vT = np.ascontiguousarray(
            adjsl.T.reshape(NB, 128, N).transpose(1, 0, 2)
        ).reshape(128, NB * N)
        hT = np.ascontiguousarray(h[b, cs].transpose(0, 2, 1)).reshape(128, N)  # [(c f), n]
        wc = w[cs]  # [2c, 4h, 64f, 64o]
        w4 = np.ascontiguousarray(wc.transpose(2, 0, 1, 3)).reshape(64, C2, NH * F)
        # wb2[f, c, hp, hr*64+o] = w[c, 2hp+hr, f, o]
        wb2 = np.ascontiguousarray(
            wc.reshape(C2, 2, 2, 64, 64).transpose(3, 0, 1, 2, 4)
        ).reshape(64, C2, 2, 128)
        # aabd[r, c, hp, col]: rows r = hr*64+f
        aabd = np.zeros((128, C2, 2, 4), np.float32)
        for c in range(C2):
            for hp in range(2):
                aabd[0:64, c, hp, 0] = a_src[cs][c, 2 * hp, :, 0]
                aabd[64:128, c, hp, 1] = a_src[cs][c, 2 * hp + 1, :, 0]
                aabd[0:64, c, hp, 2] = a_dst[cs][c, 2 * hp, :, 0]
                aabd[64:128, c, hp, 3] = a_dst[cs][c, 2 * hp + 1, :, 0]
        in_maps.append(
            {
                "vT": vT.astype(bf),
                "hTb": hT.astype(bf),
                "w4": w4.astype(bf),
                "wb2": wb2.astype(bf),
                "aabd": aabd.astype(bf),
            }
        )
    return in_maps


def kernel(h, adj, w, a_src, a_dst, trace=False):
    h = np.asarray(h, np.float32)
    adj = np.asarray(adj, np.float32)
    w = np.asarray(w, np.float32)
    a_src = np.asarray(a_src, np.float32)
    a_dst = np.asarray(a_dst, np.float32)
    nc = _build_program()
    in_maps = make_in_maps(h, adj, w, a_src, a_dst)
    if trace:
        _install_ntff_hook()
    res = run_bass_kernel_spmd(nc, in_maps, list(range(8)), trace=trace)
    out = np.zeros((4, 4, 4, N, F), np.float32)
    for core in range(8):
        b, cp = core // 2, core % 2
        out[b, 2 * cp:2 * cp + 2] = np.asarray(
            res.results[core]["out_loc"], np.float32
        )
    if trace:
        return out, res
    return out


# revision 23
# speedup vs baseline: 1.1748x; 1.1748x over previous
"""Trainium2 Bass kernel for nn_BatchMultiHeadGraphAttention.

Math: out[b,c,h] = softmax_j(mask_adj(leaky_relu(src_i + dst_j))) @ Hm where
Hm = h[b,c] @ w[c,h], t = tanh(Hm), src = t @ a_src, dst = t @ a_dst.

Key identity: exp(leaky(s)) = max(e^s, e^{0.2 s}).  The row factor
e^{0.2 src_i} cancels in softmax normalization, leaving

  P_ij  propto_i  v_ij * W_ij,   W_ij = max(sig_i * b_j, d_j)

with sig = e^{0.8 src}, b = e^{dst}, d = e^{0.2 dst}.  Per (c,h) unit the
N^2 work is exactly: 8 tensor_scalar ops (op0=mult by per-partition b_j,
op1=max with d_j) on a broadcast tile of sig, one tensor_tensor multiply
with the transposed adjacency (both DVE), and one 65-column matmul chain
num = (v .* W)^T @ [Hm | 1]; then out = num[:, :64] * (1/num[:, 64]).

The adjacency (with self-loops) is transposed/bf16-packed on the host;
all exp/tanh/matmul compute runs on device.  dst is produced directly in
column layout by small matmuls (t-blocks as lhsT); sigma rows are
broadcast across partitions via a ones-vector matmul for the first group
(latency critical) and via DRAM round-trip DMA broadcast for the rest.
Prep for group k+1 is interleaved between the mask/matmul pipelines of
units of group k; epilogues are emitted behind the next unit's masks so
the vector engine never stalls.

Sharding: core = b*2 + cpair; each core does one b and two c's (all 4
heads), data-parallel over the leading batch dim as hinted.
"""

import sys
from contextlib import ExitStack

import numpy as np
import ml_dtypes

sys.path.insert(0, "/opt/trn_rl_repo")

import concourse.bass as bass
import concourse.bacc as bacc
import concourse.tile as tile
from concourse import mybir
from concourse.bass_utils import run_bass_kernel_spmd

F32 = mybir.dt.float32
BF16 = mybir.dt.bfloat16
AF = mybir.ActivationFunctionType
OP = mybir.AluOpType

N = 1024
NB = 8  # 128-row blocks
F = 64
C2 = 2  # c's per core
NH = 4  # heads


def build_kernel(nc: bass.Bass, tc: tile.TileContext, ctx: ExitStack, ins, out_ap):
    vT_ap = ins["vT"]
    hTb_ap = ins["hTb"]
    w4_ap = ins["w4"]
    wb2_ap = ins["wb2"]
    aabd_ap = ins["aabd"]

    # ---------------- pools ----------------
    constp = ctx.enter_context(tc.tile_pool(name="const", bufs=1))
    prepp = ctx.enter_context(tc.tile_pool(name="prepp", bufs=2))
    sigbp = ctx.enter_context(tc.tile_pool(name="sigbp", bufs=8))
    vppool = ctx.enter_context(tc.tile_pool(name="vppool", bufs=3))
    vecp = ctx.enter_context(tc.tile_pool(name="vecp", bufs=4))
    outp = ctx.enter_context(tc.tile_pool(name="outp", bufs=2))
    psprep = ctx.enter_context(tc.tile_pool(name="psprep", bufs=2, space="PSUM"))
    pspp = ctx.enter_context(tc.tile_pool(name="pspp", bufs=4, space="PSUM"))

    # ---------------- constants in (small tensors first, vT last) ---------
    wb2_sb = constp.tile([64, C2, 2, 128], BF16)
    nc.sync.dma_start(out=wb2_sb[:], in_=wb2_ap[:])
    aabd_sb = constp.tile([128, C2, 2, 4], BF16)
    nc.sync.dma_start(out=aabd_sb[:], in_=aabd_ap[:])
    hTb0 = constp.tile([64, N], BF16)
    nc.sync.dma_start(out=hTb0[:, 0:512], in_=hTb_ap[0:64, 0:512])
    nc.sync.dma_start(out=hTb0[:, 512:1024], in_=hTb_ap[0:64, 512:1024])
    hTb1 = constp.tile([64, N], BF16)
    nc.sync.dma_start(out=hTb1[:], in_=hTb_ap[64:128, :])
    hTb_t = [hTb0, hTb1]
    w4_sb = constp.tile([64, C2, NH * F], BF16)
    nc.sync.dma_start(out=w4_sb[:], in_=w4_ap[:])
    vT = constp.tile([128, NB, N], BF16)
    nc.sync.dma_start(out=vT[:], in_=vT_ap[:].rearrange("p (nb n) -> p nb n", nb=NB))

    ones1 = constp.tile([1, 128], BF16)
    nc.vector.memset(ones1[:], 1.0)
    dramp = ctx.enter_context(tc.tile_pool(name="dramp", bufs=1, space="DRAM"))
    sig_d = dramp.tile([8, N], BF16, tag="sigd")
    bcols = constp.tile([128, 8, NB], F32)  # e^{dst_j}; unit index ch = c*4+h
    dcols = constp.tile([128, 8, NB], F32)  # e^{0.2 dst_j}
    sigb = {}  # per unit broadcast tiles [128, N] of e^{0.8 src_i}

    # Haug per c: [128, nb, c, h, 65] bf16 (col 64 = ones)
    haug = constp.tile([128, NB, C2, NH, 65], BF16)
    nc.vector.memset(haug[:, :, :, :, 64:65], 1.0)

    def prep_group(c, hp, pe_bcast=False):
        tT = prepp.tile([128, 2, 512], BF16, tag="tT")
        sigh = [
            prepp.tile([1, N], BF16, tag="sigh", name=f"sigh{c}{hp}0"),
            prepp.tile([1, N], BF16, tag="sigh2", name=f"sigh{c}{hp}1"),
        ]
        sbs = [
            sigbp.tile([128, N], BF16, tag="sigb", name=f"sigb{c*4+2*hp}"),
            sigbp.tile([128, N], BF16, tag="sigb", name=f"sigb{c*4+2*hp+1}"),
        ]
        for half in range(2):
            hs = slice(half * 512, (half + 1) * 512)
            ps_t = psprep.tile([128, 512], F32, tag="prep")
            nc.tensor.matmul(
                ps_t[:],
                lhsT=wb2_sb[:, c, hp, :],
                rhs=hTb_t[c][:, hs],
                start=True,
                stop=True,
            )
            nc.scalar.activation(out=tT[:, half, :], in_=ps_t[:], func=AF.Tanh)
        # dst columns directly via matmul: psd[:, jb*2+hh] = dst^{head hh}_j
        psd = psprep.tile([128, 16], F32, tag="psd")
        for jb in range(NB):
            half, q = jb // 4, jb % 4
            nc.tensor.matmul(
                psd[:, jb * 2:jb * 2 + 2],
                lhsT=tT[:, half, q * 128:(q + 1) * 128],
                rhs=aabd_sb[:, c, hp, 2:4],
                start=True,
                stop=True,
            )
        for hh in range(2):
            ch = c * 4 + 2 * hp + hh
            for half in range(2):
                hs = slice(half * 512, (half + 1) * 512)
                ps_src = psprep.tile([128, 512], F32, tag="prep")
                nc.tensor.matmul(
                    ps_src[0:1, :],
                    lhsT=aabd_sb[:, c, hp, hh:hh + 1],
                    rhs=tT[:, half, :],
                    start=True,
                    stop=True,
                )
                nc.scalar.activation(
                    out=sigh[hh][:, hs], in_=ps_src[0:1, :], func=AF.Exp, scale=0.8
                )
                if pe_bcast:
                    # broadcast sigma row across partitions on the PE
                    psb = psprep.tile([128, 512], F32, tag="prep")
                    nc.tensor.matmul(
                        psb[:],
                        lhsT=ones1[:],
                        rhs=sigh[hh][:, hs],
                        start=True,
                        stop=True,
                    )
                    nc.scalar.activation(
                        out=sbs[hh][:, hs], in_=psb[:], func=AF.Copy
                    )
            dview = psd[:, 0:16].rearrange("p (jb h) -> p h jb", h=2)[:, hh, :]
            nc.scalar.activation(out=bcols[:, ch, :], in_=dview, func=AF.Exp)
            nc.scalar.activation(
                out=dcols[:, ch, :], in_=dview, func=AF.Exp, scale=0.2
            )
            if not pe_bcast:
                nc.sync.dma_start(out=sig_d[ch:ch + 1, :], in_=sigh[hh][:])
                nc.gpsimd.dma_start(
                    out=sbs[hh][:],
                    in_=sig_d[ch:ch + 1, :].to_broadcast([128, N]),
                )
            sigb[ch] = sbs[hh]

    def prep_haug(c):
        for nb in range(NB):
            ph = psprep.tile([128, 512], F32, tag="prep")
            nc.tensor.matmul(
                ph[:, 0:256],
                lhsT=hTb_t[c][:, nb * 128:(nb + 1) * 128],
                rhs=w4_sb[:, c, :],
                start=True,
                stop=True,
            )
            nc.scalar.activation(
                out=haug[:, nb, c, :, 0:64],
                in_=ph[:, 0:256].rearrange("p (h o) -> p h o", h=NH),
                func=AF.Copy,
            )


    # ---------------- N^2 phase, software-pipelined ----------------
    UNITS = [(0, 0), (0, 1), (1, 0), (1, 1), (0, 2), (0, 3), (1, 2), (1, 3)]

    state = {}

    def masks(idx):
        c, h = UNITS[idx]
        ch = c * 4 + h
        W = vppool.tile([128, NB, N], BF16, tag="wv", name=f"wv{ch}")
        for jb in range(NB):
            nc.vector.tensor_scalar(
                out=W[:, jb, :],
                in0=sigb[ch][:],
                scalar1=bcols[:, ch, jb:jb + 1],
                scalar2=dcols[:, ch, jb:jb + 1],
                op0=OP.mult,
                op1=OP.max,
            )
        if idx == len(UNITS) - 1:
            for ihalf in range(2):
                cols = slice(ihalf * 512, (ihalf + 1) * 512)
                nc.vector.tensor_tensor(
                    out=W[:, :, cols], in0=W[:, :, cols], in1=vT[:, :, cols],
                    op=OP.mult,
                )
        else:
            nc.vector.tensor_tensor(out=W[:], in0=W[:], in1=vT[:], op=OP.mult)
        state[idx] = W

    def chains(idx):
        c, h = UNITS[idx]
        W = state[idx]
        pss = []
        for ihalf in range(2):
            ps_n = pspp.tile([128, 4, 65], F32, tag="psn")
            for q in range(4):
                ib = ihalf * 4 + q
                for jb in range(NB):
                    nc.tensor.matmul(
                        ps_n[:, q, :],
                        lhsT=W[:, jb, ib * 128:(ib + 1) * 128],
                        rhs=haug[:, jb, c, h, :],
                        start=(jb == 0),
                        stop=(jb == NB - 1),
                    )
            pss.append(ps_n)
        state[idx] = (pss, c, h)

    def epilogue(idx):
        pss, c, h = state.pop(idx)
        outb = outp.tile([128, NB, F], BF16, tag="outb")
        for ihalf in range(2):
            ps_n = pss[ihalf]
            rec = vecp.tile([128, 4, 1], F32, tag="rec")
            nc.vector.reciprocal(out=rec[:], in_=ps_n[:, :, 64:65])
            for q in range(4):
                ib = ihalf * 4 + q
                nc.scalar.activation(
                    out=outb[:, ib, :],
                    in_=ps_n[:, q, 0:64],
                    func=AF.Copy,
                    scale=rec[:, q, :],
                )
        for ihalf in range(2):
            nc.sync.dma_start(
                out=out_ap[c, h, ihalf * 512:(ihalf + 1) * 512, :].rearrange(
                    "(ib p) o -> p ib o", p=128
                ),
                in_=outb[:, ihalf * 4:(ihalf + 1) * 4, :],
            )

    prep_group(0, 0, pe_bcast=True)
    masks(0)
    prep_haug(0)
    prep_group(1, 0, pe_bcast=True)
    chains(0)
    masks(1)
    prep_haug(1)
    chains(1)
    prep_group(0, 1)
    masks(2)
    chains(2)
    epilogue(0)
    prep_group(1, 1)
    for idx in range(3, len(UNITS) - 1):
        masks(idx)
        chains(idx)
        epilogue(idx - 2)
    masks(7)
    epilogue(5)
    epilogue(6)
    chains(7)
    epilogue(7)


def _install_ntff_hook():
    """antenv.axon_hooks is missing in this image; inject an equivalent shim
    driving NTFF profiling via ctypes into libaxon_pjrt.so."""
    import types, ctypes, contextlib

    if "antenv.axon_hooks" in sys.modules:
        return
    so_path = "/opt/axon/libaxon_pjrt.so"
    try:
        lib = ctypes.CDLL(so_path)
        lib.axon_start_nrt_profile.argtypes = [
            ctypes.POINTER(ctypes.c_int64),
            ctypes.c_size_t,
        ]
        lib.axon_start_nrt_profile.restype = ctypes.c_int64
        lib.axon_stop_nrt_profile.argtypes = [ctypes.c_char_p]
        lib.axon_stop_nrt_profile.restype = ctypes.c_int64
    except (OSError, AttributeError):
        return

    @contextlib.contextmanager
    def _hook(output_dir, device_ids):
        import jax

        jax.devices()
        if device_ids:
            ids = (ctypes.c_int64 * len(device_ids))(*device_ids)
            rc = lib.axon_start_nrt_profile(ids, len(device_ids))
        else:
            rc = lib.axon_start_nrt_profile(None, 0)
        if rc != 0:
            raise RuntimeError(f"axon_start_nrt_profile rc={rc}")
        try:
            yield
        finally:
            n = lib.axon_stop_nrt_profile(str(output_dir).encode())
            print(f"profile: {n} file(s) written to {output_dir}", file=sys.stderr)

    mod = types.ModuleType("antenv.axon_hooks")
    mod.get_axon_ntff_profile_hook = lambda: _hook
    mod.set_axon_ntff_profile_hook = lambda h: None
    sys.modules["antenv.axon_hooks"] = mod

    import concourse.bass_utils as bu

    bu.upload_artifacts = lambda tmpdir: f"local:{tmpdir}"


_CACHED = {}


def _build_program():
    if "nc" in _CACHED:
        return _CACHED["nc"]
    nc = bacc.Bacc(
        "TRN2",
        target_bir_lowering=False,
        debug=False,
        enable_asserts=True,
        num_devices=8,
    )
    ins = {
        "vT": nc.dram_tensor("vT", [128, NB * N], BF16, kind="ExternalInput").ap(),
        "hTb": nc.dram_tensor("hTb", [128, N], BF16, kind="ExternalInput").ap(),
        "w4": nc.dram_tensor("w4", [64, C2, NH * F], BF16, kind="ExternalInput").ap(),
        "wb2": nc.dram_tensor("wb2", [64, C2, 2, 128], BF16, kind="ExternalInput").ap(),
        "aabd": nc.dram_tensor("aabd", [128, C2, 2, 4], BF16, kind="ExternalInput").ap(),
    }
    out_ap = nc.dram_tensor(
        "out_loc", [C2, NH, N, F], BF16, kind="ExternalOutput"
    ).ap()
    with tile.TileContext(nc) as tc:
        with ExitStack() as ctx:
            build_kernel(nc, tc, ctx, ins, out_ap)
    nc.compile()
    _CACHED["nc"] = nc
    return nc


def make_in_maps(h, adj, w, a_src, a_dst):
    bf = ml_dtypes.bfloat16
    in_maps = []
    eye = np.eye(N, dtype=np.float32)
    for core in range(8):
        b, cp = core // 2, core % 2
        cs = slice(2 * cp, 2 * cp + 2)
        # vT[p, jb, i] = (adj[b] or I)[i, jb*128+p]
        adjsl = ((adj[b] + eye) != 0).astype(np.float32)
        vT = np.ascontiguousarray(
            adjsl.T.reshape(NB, 128, N).transpose(1, 0, 2)
        ).reshape(128, NB * N)
        hT = np.ascontiguousarray(h[b, cs].transpose(0, 2, 1)).reshape(128, N)  # [(c f), n]
        wc = w[cs]  # [2c, 4h, 64f, 64o]
        w4 = np.ascontiguousarray(wc.transpose(2, 0, 1, 3)).reshape(64, C2, NH * F)
        # wb2[f, c, hp, hr*64+o] = w[c, 2hp+hr, f, o]
        wb2 = np.ascontiguousarray(
            wc.reshape(C2, 2, 2, 64, 64).transpose(3, 0, 1, 2, 4)
        ).reshape(64, C2, 2, 128)
        # aabd[r, c, hp, col]: rows r = hr*64+f
        aabd = np.zeros((128, C2, 2, 4), np.float32)
        for c in range(C2):
            for hp in range(2):
                aabd[0:64, c, hp, 0] = a_src[cs][c, 2 * hp, :, 0]
                aabd[64:128, c, hp, 1] = a_src[cs][c, 2 * hp + 1, :, 0]
                aabd[0:64, c, hp, 2] = a_dst[cs][c, 2 * hp, :, 0]
                aabd[64:128, c, hp, 3] = a_dst[cs][c, 2 * hp + 1, :, 0]
        in_maps.append(
            {
                "vT": vT.astype(bf),
                "hTb": hT.astype(bf),
                "w4": w4.astype(bf),
                "wb2": wb2.astype(bf),
                "aabd": aabd.astype(bf),
            }
        )
    return in_maps


def kernel(h, adj, w, a_src, a_dst, trace=False):
    h = np.asarray(h, np.float32)
    adj = np.asarray(adj, np.float32)
    w = np.asarray(w, np.float32)
    a_src = np.asarray(a_src, np.float32)
    a_dst = np.asarray(a_dst, np.float32)
    nc = _build_program()
    in_maps = make_in_maps(h, adj, w, a_src, a_dst)
    if trace:
        _install_ntff_hook()
    res = run_bass_kernel_spmd(nc, in_maps, list(range(8)), trace=trace)
    out = np.zeros((4, 4, 4, N, F), np.float32)
    for core in range(8):
        b, cp = core // 2, core % 2
        out[b, 2 * cp:2 * cp + 2] = np.asarray(
            res.results[core]["out_loc"], np.float32
        )
    if trace:
        return out, res
    return out


# revision 24
# speedup vs baseline: 1.2132x; 1.0326x over previous
"""Trainium2 Bass kernel for nn_BatchMultiHeadGraphAttention.

Math: out[b,c,h] = softmax_j(mask_adj(leaky_relu(src_i + dst_j))) @ Hm where
Hm = h[b,c] @ w[c,h], t = tanh(Hm), src = t @ a_src, dst = t @ a_dst.

Key identity: exp(leaky(s)) = max(e^s, e^{0.2 s}).  The row factor
e^{0.2 src_i} cancels in softmax normalization, leaving

  P_ij  propto_i  v_ij * W_ij,   W_ij = max(sig_i * b_j, d_j)

with sig = e^{0.8 src}, b = e^{dst}, d = e^{0.2 dst}.  Per (c,h) unit the
N^2 work is exactly: 8 tensor_scalar ops (op0=mult by per-partition b_j,
op1=max with d_j) on a broadcast tile of sig, one tensor_tensor multiply
with the transposed adjacency (both DVE), and one 65-column matmul chain
num = (v .* W)^T @ [Hm | 1]; then out = num[:, :64] * (1/num[:, 64]).

The adjacency (with self-loops) is transposed/bf16-packed on the host;
all exp/tanh/matmul compute runs on device.  dst is produced directly in
column layout by small matmuls (t-blocks as lhsT); sigma rows are
broadcast across partitions via a ones-vector matmul for the first group
(latency critical) and via DRAM round-trip DMA broadcast for the rest.
Prep for group k+1 is interleaved between the mask/matmul pipelines of
units of group k; epilogues are emitted behind the next unit's masks so
the vector engine never stalls.

Sharding: core = b*2 + cpair; each core does one b and two c's (all 4
heads), data-parallel over the leading batch dim as hinted.
"""

import sys
from contextlib import ExitStack

import numpy as np
import ml_dtypes

sys.path.insert(0, "/opt/trn_rl_repo")

import concourse.bass as bass
import concourse.bacc as bacc
import concourse.tile as tile
from concourse import mybir
from concourse.bass_utils import run_bass_kernel_spmd

F32 = mybir.dt.float32
BF16 = mybir.dt.bfloat16
AF = mybir.ActivationFunctionType
OP = mybir.AluOpType

N = 1024
NB = 8  # 128-row blocks
F = 64
C2 = 2  # c's per core
NH = 4  # heads


def build_kernel(nc: bass.Bass, tc: tile.TileContext, ctx: ExitStack, ins, out_ap):
    vT_ap = ins["vT"]
    hTb_ap = ins["hTb"]
    w4_ap = ins["w4"]
    wb2_ap = ins["wb2"]
    aabd_ap = ins["aabd"]

    # ---------------- pools ----------------
    constp = ctx.enter_context(tc.tile_pool(name="const", bufs=1))
    prepp = ctx.enter_context(tc.tile_pool(name="prepp", bufs=2))
    sigbp = ctx.enter_context(tc.tile_pool(name="sigbp", bufs=8))
    vppool = ctx.enter_context(tc.tile_pool(name="vppool", bufs=3))
    vecp = ctx.enter_context(tc.tile_pool(name="vecp", bufs=4))
    outp = ctx.enter_context(tc.tile_pool(name="outp", bufs=2))
    psprep = ctx.enter_context(tc.tile_pool(name="psprep", bufs=2, space="PSUM"))
    pspp = ctx.enter_context(tc.tile_pool(name="pspp", bufs=4, space="PSUM"))

    # ---------------- constants in (small tensors first, vT last) ---------
    wb2_sb = constp.tile([64, C2, 2, 128], BF16)
    nc.sync.dma_start(out=wb2_sb[:], in_=wb2_ap[:])
    aabd_sb = constp.tile([128, C2, 2, 4], BF16)
    nc.sync.dma_start(out=aabd_sb[:], in_=aabd_ap[:])
    hTb0 = constp.tile([64, N], BF16)
    nc.sync.dma_start(out=hTb0[:, 0:512], in_=hTb_ap[0:64, 0:512])
    nc.sync.dma_start(out=hTb0[:, 512:1024], in_=hTb_ap[0:64, 512:1024])
    hTb1 = constp.tile([64, N], BF16)
    nc.sync.dma_start(out=hTb1[:], in_=hTb_ap[64:128, :])
    hTb_t = [hTb0, hTb1]
    w4_sb = constp.tile([64, C2, NH * F], BF16)
    nc.sync.dma_start(out=w4_sb[:], in_=w4_ap[:])
    vT = constp.tile([128, NB, N], BF16)
    nc.sync.dma_start(out=vT[:], in_=vT_ap[:].rearrange("p (nb n) -> p nb n", nb=NB))

    ones1 = constp.tile([1, 128], BF16)
    nc.vector.memset(ones1[:], 1.0)
    dramp = ctx.enter_context(tc.tile_pool(name="dramp", bufs=1, space="DRAM"))
    sig_d = dramp.tile([8, N], BF16, tag="sigd")
    bcols = constp.tile([128, 8, NB], F32)  # e^{dst_j}; unit index ch = c*4+h
    dcols = constp.tile([128, 8, NB], F32)  # e^{0.2 dst_j}
    sigb = {}  # per unit broadcast tiles [128, N] of e^{0.8 src_i}

    # Haug per c: [128, nb, c, h, 65] bf16 (col 64 = ones)
    haug = constp.tile([128, NB, C2, NH, 65], BF16)
    nc.vector.memset(haug[:, :, :, :, 64:65], 1.0)

    def prep_group(c, hp, pe_bcast=False):
        tT = prepp.tile([128, 2, 512], BF16, tag="tT")
        sigh = [
            prepp.tile([1, N], BF16, tag="sigh", name=f"sigh{c}{hp}0"),
            prepp.tile([1, N], BF16, tag="sigh2", name=f"sigh{c}{hp}1"),
        ]
        sbs = [
            sigbp.tile([128, N], BF16, tag="sigb", name=f"sigb{c*4+2*hp}"),
            sigbp.tile([128, N], BF16, tag="sigb", name=f"sigb{c*4+2*hp+1}"),
        ]
        for half in range(2):
            hs = slice(half * 512, (half + 1) * 512)
            ps_t = psprep.tile([128, 512], F32, tag="prep")
            nc.tensor.matmul(
                ps_t[:],
                lhsT=wb2_sb[:, c, hp, :],
                rhs=hTb_t[c][:, hs],
                start=True,
                stop=True,
            )
            nc.scalar.activation(out=tT[:, half, :], in_=ps_t[:], func=AF.Tanh)
        # dst columns directly via matmul: psd[:, jb*2+hh] = dst^{head hh}_j
        psd = psprep.tile([128, 16], F32, tag="psd")
        for jb in range(NB):
            half, q = jb // 4, jb % 4
            nc.tensor.matmul(
                psd[:, jb * 2:jb * 2 + 2],
                lhsT=tT[:, half, q * 128:(q + 1) * 128],
                rhs=aabd_sb[:, c, hp, 2:4],
                start=True,
                stop=True,
            )
        for hh in range(2):
            ch = c * 4 + 2 * hp + hh
            for half in range(2):
                hs = slice(half * 512, (half + 1) * 512)
                ps_src = psprep.tile([128, 512], F32, tag="prep")
                nc.tensor.matmul(
                    ps_src[0:1, :],
                    lhsT=aabd_sb[:, c, hp, hh:hh + 1],
                    rhs=tT[:, half, :],
                    start=True,
                    stop=True,
                )
                nc.scalar.activation(
                    out=sigh[hh][:, hs], in_=ps_src[0:1, :], func=AF.Exp, scale=0.8
                )
                if pe_bcast:
                    # broadcast sigma row across partitions on the PE
                    psb = psprep.tile([128, 512], F32, tag="prep")
                    nc.tensor.matmul(
                        psb[:],
                        lhsT=ones1[:],
                        rhs=sigh[hh][:, hs],
                        start=True,
                        stop=True,
                    )
                    nc.scalar.activation(
                        out=sbs[hh][:, hs], in_=psb[:], func=AF.Copy
                    )
            dview = psd[:, 0:16].rearrange("p (jb h) -> p h jb", h=2)[:, hh, :]
            nc.scalar.activation(out=bcols[:, ch, :], in_=dview, func=AF.Exp)
            nc.scalar.activation(
                out=dcols[:, ch, :], in_=dview, func=AF.Exp, scale=0.2
            )
            if not pe_bcast:
                nc.sync.dma_start(out=sig_d[ch:ch + 1, :], in_=sigh[hh][:])
                nc.gpsimd.dma_start(
                    out=sbs[hh][:],
                    in_=sig_d[ch:ch + 1, :].to_broadcast([128, N]),
                )
            sigb[ch] = sbs[hh]

    def prep_haug(c):
        for nb in range(NB):
            ph = psprep.tile([128, 512], F32, tag="prep")
            nc.tensor.matmul(
                ph[:, 0:256],
                lhsT=hTb_t[c][:, nb * 128:(nb + 1) * 128],
                rhs=w4_sb[:, c, :],
                start=True,
                stop=True,
            )
            nc.scalar.activation(
                out=haug[:, nb, c, :, 0:64],
                in_=ph[:, 0:256].rearrange("p (h o) -> p h o", h=NH),
                func=AF.Copy,
            )


    # ---------------- N^2 phase, software-pipelined ----------------
    UNITS = [(0, 0), (0, 1), (1, 0), (1, 1), (0, 2), (0, 3), (1, 2), (1, 3)]

    state = {}

    def masks(idx):
        c, h = UNITS[idx]
        ch = c * 4 + h
        W = vppool.tile([128, NB, N], BF16, tag="wv", name=f"wv{ch}")
        for jb in range(NB):
            nc.vector.tensor_scalar(
                out=W[:, jb, :],
                in0=sigb[ch][:],
                scalar1=bcols[:, ch, jb:jb + 1],
                scalar2=dcols[:, ch, jb:jb + 1],
                op0=OP.mult,
                op1=OP.max,
            )
        if idx == len(UNITS) - 1:
            for ihalf in range(2):
                cols = slice(ihalf * 512, (ihalf + 1) * 512)
                nc.vector.tensor_tensor(
                    out=W[:, :, cols], in0=W[:, :, cols], in1=vT[:, :, cols],
                    op=OP.mult,
                )
        else:
            nc.vector.tensor_tensor(out=W[:], in0=W[:], in1=vT[:], op=OP.mult)
        state[idx] = W

    def chains(idx):
        c, h = UNITS[idx]
        W = state[idx]
        pss = []
        for ihalf in range(2):
            ps_n = pspp.tile([128, 4, 65], F32, tag="psn")
            for q in range(4):
                ib = ihalf * 4 + q
                for jb in range(NB):
                    nc.tensor.matmul(
                        ps_n[:, q, :],
                        lhsT=W[:, jb, ib * 128:(ib + 1) * 128],
                        rhs=haug[:, jb, c, h, :],
                        start=(jb == 0),
                        stop=(jb == NB - 1),
                    )
            pss.append(ps_n)
        state[idx] = (pss, c, h)

    def epilogue(idx):
        pss, c, h = state.pop(idx)
        outb = outp.tile([128, NB, F], BF16, tag="outb")
        for ihalf in range(2):
            ps_n = pss[ihalf]
            rec = vecp.tile([128, 4, 1], F32, tag="rec")
            nc.vector.reciprocal(out=rec[:], in_=ps_n[:, :, 64:65])
            for q in range(4):
                ib = ihalf * 4 + q
                nc.scalar.activation(
                    out=outb[:, ib, :],
                    in_=ps_n[:, q, 0:64],
                    func=AF.Copy,
                    scale=rec[:, q, :],
                )
        for ihalf in range(2):
            nc.sync.dma_start(
                out=out_ap[c, h, ihalf * 512:(ihalf + 1) * 512, :].rearrange(
                    "(ib p) o -> p ib o", p=128
                ),
                in_=outb[:, ihalf * 4:(ihalf + 1) * 4, :],
            )

    prep_group(0, 0, pe_bcast=True)
    masks(0)
    prep_haug(0)
    prep_group(1, 0, pe_bcast=True)
    chains(0)
    masks(1)
    prep_haug(1)
    chains(1)
    prep_group(0, 1)
    masks(2)
    chains(2)
    epilogue(0)
    prep_group(1, 1)
    for idx in range(3, len(UNITS) - 1):
        masks(idx)
        chains(idx)
        epilogue(idx - 2)
    epilogue(5)
    masks(7)
    epilogue(6)
    chains(7)
    epilogue(7)


def _install_ntff_hook():
    """antenv.axon_hooks is missing in this image; inject an equivalent shim
    driving NTFF profiling via ctypes into libaxon_pjrt.so."""
    import types, ctypes, contextlib

    if "antenv.axon_hooks" in sys.modules:
        return
    so_path = "/opt/axon/libaxon_pjrt.so"
    try:
        lib = ctypes.CDLL(so_path)
        lib.axon_start_nrt_profile.argtypes = [
            ctypes.POINTER(ctypes.c_int64),
            ctypes.c_size_t,
        ]
        lib.axon_start_nrt_profile.restype = ctypes.c_int64
        lib.axon_stop_nrt_profile.argtypes = [ctypes.c_char_p]
        lib.axon_stop_nrt_profile.restype = ctypes.c_int64
    except (OSError, AttributeError):
        return

    @contextlib.contextmanager
    def _hook(output_dir, device_ids):
        import jax

        jax.devices()
        if device_ids:
            ids = (ctypes.c_int64 * len(device_ids))(*device_ids)
            rc = lib.axon_start_nrt_profile(ids, len(device_ids))
        else:
            rc = lib.axon_start_nrt_profile(None, 0)
        if rc != 0:
            raise RuntimeError(f"axon_start_nrt_profile rc={rc}")
        try:
            yield
        finally:
            n = lib.axon_stop_nrt_profile(str(output_dir).encode())
            print(f"profile: {n} file(s) written to {output_dir}", file=sys.stderr)

    mod = types.ModuleType("antenv.axon_hooks")
    mod.get_axon_ntff_profile_hook = lambda: _hook
    mod.set_axon_ntff_profile_hook = lambda h: None
    sys.modules["antenv.axon_hooks"] = mod

    import concourse.bass_utils as bu

    bu.upload_artifacts = lambda tmpdir: f"local:{tmpdir}"


_CACHED = {}


def _build_program():
    if "nc" in _CACHED:
        return _CACHED["nc"]
    nc = bacc.Bacc(
        "TRN2",
        target_bir_lowering=False,
        debug=False,
        enable_asserts=True,
        num_devices=8,
    )
    ins = {
        "vT": nc.dram_tensor("vT", [128, NB * N], BF16, kind="ExternalInput").ap(),
        "hTb": nc.dram_tensor("hTb", [128, N], BF16, kind="ExternalInput").ap(),
        "w4": nc.dram_tensor("w4", [64, C2, NH * F], BF16, kind="ExternalInput").ap(),
        "wb2": nc.dram_tensor("wb2", [64, C2, 2, 128], BF16, kind="ExternalInput").ap(),
        "aabd": nc.dram_tensor("aabd", [128, C2, 2, 4], BF16, kind="ExternalInput").ap(),
    }
    out_ap = nc.dram_tensor(
        "out_loc", [C2, NH, N, F], BF16, kind="ExternalOutput"
    ).ap()
    with tile.TileContext(nc) as tc:
        with ExitStack() as ctx:
            build_kernel(nc, tc, ctx, ins, out_ap)
    nc.compile()
    _CACHED["nc"] = nc
    return nc


def make_in_maps(h, adj, w, a_src, a_dst):
    bf = ml_dtypes.bfloat16
    in_maps = []
    eye = np.eye(N, dtype=np.float32)
    for core in range(8):
        b, cp = core // 2, core % 2
        cs = slice(2 * cp, 2 * cp + 2)
        # vT[p, jb, i] = (adj[b] or I)[i, jb*128+p]
        adjsl = ((adj[b] + eye) != 0).astype(np.float32)
        vT = np.ascontiguousarray(
            adjsl.T.reshape(NB, 128, N).transpose(1, 0, 2)
        ).reshape(128, NB * N)
        hT = np.ascontiguousarray(h[b, cs].transpose(0, 2, 1)).reshape(128, N)  # [(c f), n]
        wc = w[cs]  # [2c, 4h, 64f, 64o]
        w4 = np.ascontiguousarray(wc.transpose(2, 0, 1, 3)).reshape(64, C2, NH * F)
        # wb2[f, c, hp, hr*64+o] = w[c, 2hp+hr, f, o]
        wb2 = np.ascontiguousarray(
            wc.reshape(C2, 2, 2, 64, 64).transpose(3, 0, 1, 2, 4)
        ).reshape(64, C2, 2, 128)
        # aabd[r, c, hp, col]: rows r = hr*64+f
        aabd = np.zeros((128, C2, 2, 4), np.float32)
        for c in range(C2):
            for hp in range(2):
                aabd[0:64, c, hp, 0] = a_src[cs][c, 2 * hp, :, 0]
                aabd[64:128, c, hp, 1] = a_src[cs][c, 2 * hp + 1, :, 0]
                aabd[0:64, c, hp, 2] = a_dst[cs][c, 2 * hp, :, 0]
                aabd[64:128, c, hp, 3] = a_dst[cs][c, 2 * hp + 1, :, 0]
        in_maps.append(
            {
                "vT": vT.astype(bf),
                "hTb": hT.astype(bf),
                "w4": w4.astype(bf),
                "wb2": wb2.astype(bf),
                "aabd": aabd.astype(bf),
            }
        )
    return in_maps


def kernel(h, adj, w, a_src, a_dst, trace=False):
    h = np.asarray(h, np.float32)
    adj = np.asarray(adj, np.float32)
    w = np.asarray(w, np.float32)
    a_src = np.asarray(a_src, np.float32)
    a_dst = np.asarray(a_dst, np.float32)
    nc = _build_program()
    in_maps = make_in_maps(h, adj, w, a_src, a_dst)
    if trace:
        _install_ntff_hook()
    res = run_bass_kernel_spmd(nc, in_maps, list(range(8)), trace=trace)
    out = np.zeros((4, 4, 4, N, F), np.float32)
    for core in range(8):
        b, cp = core // 2, core % 2
        out[b, 2 * cp:2 * cp + 2] = np.asarray(
            res.results[core]["out_loc"], np.float32
        )
    if trace:
        return out, res
    return out
